# revision 29
# baseline (speedup 1.0000x reference)
"""Trainium2 Bass kernel for nn_AttnAggregator (GNN message passing, 8 cores).

Data-parallel over queries: each of 8 NeuronCores owns 256 queries = 2560
segments = 20 windows of 128 segments. Neighbor lists per window are padded
to 128-slot tiles; each core processes its windows sorted by tile count
(descending) so the SPMD-uniform per-position tile count T_j = max over
cores of similarly-ranked windows; the host unpermutes output rows.

Device pipeline per tile (128 slots), all layouts pre-gathered on host:
  zT[h',slot] = W1.T @ em       PE fp8 DoubleRow (K=256 in one pass; one
                                call per quad of 4 tiles per h'-half)
  zT += C[q(slot)]              PE fp8 one-hot bias rows, k-padded to 128
                                partitions (c as hi+lo fp8 rows for
                                ~fp16 bias precision)
  HT = tanh(zT)                 ACT, one call per quad, fp16 out
  score[slot] = HT.T @ v        PE fp16 (2 k-half calls into a PSUM col)
  e = exp(score)                ACT per window, from score PSUM
  wm = P8 * e                   one batched DVE tensor_tensor per window
                                (P8 = shipped one-hot skeleton, e bcast
                                via stride-0 AP), fp16 out
  agg|den += wm.T @ [em16 | 1]  PE fp16, accumulated across the window
  out16 = agg|den               DVE copy psum->fp16, DMA out

Host: input layout + gather (as before), then assemble: divide agg/den,
unpermute windows, and fill the broadcast s_emb/r_emb output columns
(pure data movement; the baseline shipped them through the device as a
DRAM->DRAM copy). The agg/score software-pipeline lags one window so the
PE never waits on the exp->wm chain.
"""

import os
import sys

import numpy as np

H = 256
SEQ_LEN = 10
NCORES = 8
WIN = 128  # segments per window (PSUM partition dim)
QMAX = 16  # bias one-hot rows per window (max 14 queries touch a window)


def _build_core_shard(c, nbr_ids, seg_ids, QPC, NW):
    seg_lo = c * QPC * SEQ_LEN
    seg_hi = (c + 1) * QPC * SEQ_LEN
    lo = np.searchsorted(seg_ids, seg_lo, "left")
    hi = np.searchsorted(seg_ids, seg_hi, "left")
    segs = (seg_ids[lo:hi] - seg_lo).astype(np.int64)  # 0 .. SPC-1
    nbrs = nbr_ids[lo:hi].astype(np.int64)
    wb = [np.searchsorted(segs, w * WIN, "left") for w in range(NW + 1)]
    cnts = [wb[w + 1] - wb[w] for w in range(NW)]
    return segs, nbrs, wb, cnts


def kernel(s, r, nbr_ids, seg_ids, ent_embeds, rel_embeds, W_attn, b_attn, v_s):
    sys.path.insert(0, "/opt/trn_rl_repo")
    import ml_dtypes
    import concourse.bass as bass  # noqa: F401
    import concourse.tile as tile
    from concourse import bacc, mybir
    from concourse.bass_utils import run_bass_kernel_spmd
    from contextlib import ExitStack

    f32 = mybir.dt.float32
    f16 = mybir.dt.float16
    f8 = mybir.dt.float8e4
    AF = mybir.ActivationFunctionType
    OP = mybir.AluOpType
    DR = mybir.MatmulPerfMode.DoubleRow
    np8 = ml_dtypes.float8_e4m3

    s = np.asarray(s)
    r = np.asarray(r)
    nbr_ids = np.asarray(nbr_ids)
    seg_ids = np.asarray(seg_ids)
    ent_embeds = np.ascontiguousarray(np.asarray(ent_embeds, dtype=np.float32))
    rel_embeds = np.ascontiguousarray(np.asarray(rel_embeds, dtype=np.float32))
    W_attn = np.asarray(W_attn, dtype=np.float32)
    b_attn = np.asarray(b_attn, dtype=np.float32)
    v_s = np.asarray(v_s, dtype=np.float32).reshape(-1)

    B = s.shape[0]
    NUM_SEG = B * SEQ_LEN
    QPC = B // NCORES
    SPC = QPC * SEQ_LEN
    NW = SPC // WIN

    W1 = W_attn[0:256]
    c_all = ent_embeds[s] @ W_attn[256:512] + rel_embeds[r] @ W_attn[512:768] \
        + b_attn                                     # [B, 256]
    c_hi = c_all.astype(np8).astype(np.float32)
    c_lo = (c_all - c_hi).astype(np8).astype(np.float32)

    # fp8 hi/lo entity tables (hi also serves as the z-matmul operand)
    ent_hi8 = ent_embeds.astype(np8)
    ent_lo8 = (ent_embeds - ent_hi8.astype(np.float32)).astype(np8)

    # ---------------- host-side layout ----------------
    shards = [_build_core_shard(c, nbr_ids, seg_ids, QPC, NW) for c in range(NCORES)]
    tc_cw = np.array(
        [[max(1, -(-shards[c][3][w] // 128)) for w in range(NW)]
         for c in range(NCORES)])
    perm = [list(np.argsort(-tc_cw[c], kind="stable")) for c in range(NCORES)]
    T_j = [int(max(tc_cw[c][perm[c][j]] for c in range(NCORES)))
           for j in range(NW)]
    Q_j = [-(-t // 4) for t in T_j]                 # quads per window
    P_j = [-(-t // 2) for t in T_j]                 # pairs per window
    tb = np.concatenate([[0], np.cumsum(T_j)]).astype(np.int64)
    qb = np.concatenate([[0], np.cumsum(Q_j)]).astype(np.int64)
    pb = np.concatenate([[0], np.cumsum(P_j)]).astype(np.int64)
    NT = int(tb[-1])
    QT = int(qb[-1])
    NTP = int(pb[-1])
    TMAX = max(T_j)
    QMAXW = max(Q_j)
    PMAXW = max(P_j)

    counts_all = np.bincount(seg_ids.astype(np.int64), minlength=NUM_SEG)

    in_maps = []
    for c in range(NCORES):
        segs, nbrs, wb, cnts = shards[c]

        em_idx = np.full((NT, 128), -1, dtype=np.int64)
        segl = np.full((NT, 128), 255, dtype=np.int64)
        qloc = np.full((NT, 128), -1, dtype=np.int64)
        qbase = np.zeros(NW, dtype=np.int64)  # per window POSITION j

        for j in range(NW):
            w = perm[c][j]
            cnt = cnts[w]
            flat_lo = int(tb[j]) * 128
            sl = slice(wb[w], wb[w + 1])
            idx_flat = np.arange(flat_lo, flat_lo + cnt)
            em_idx.reshape(-1)[idx_flat] = nbrs[sl]
            segl.reshape(-1)[idx_flat] = segs[sl] - w * WIN
            qb_w = (w * WIN) // SEQ_LEN
            qbase[j] = qb_w
            qloc.reshape(-1)[idx_flat] = segs[sl] // SEQ_LEN - qb_w

        valid = em_idx >= 0
        idx = np.maximum(em_idx, 0)
        Ehi = ent_hi8[idx]                         # [NT, 128, 256] fp8
        Elo = ent_lo8[idx]
        Ehi[~valid] = 0
        Elo[~valid] = 0

        # em16 [128, NT, 257]: [slot, tile, col]; ones col for den
        E16 = ent_embeds.astype(np.float16)[idx]
        E16[~valid] = 0
        em16 = np.zeros((128, NT, 257), dtype=np.float16)
        em16[:, :, 0:256] = E16.transpose(1, 0, 2)
        em16[:, :, 256] = np.float16(1.0)

        # emT8 quad-major [128, QT, 2, 512]:
        #   [p, qd, i, qs] = em[tile = quad_tile(qd)+qs//128, slot qs%128,
        #                       h = i*128+p]
        emT8 = np.zeros((128, QT, 2, 512), dtype=np8)
        q1hot = np.zeros((16, QT, 512), dtype=np8)
        for j in range(NW):
            for qd in range(Q_j[j]):
                g = int(qb[j]) + qd
                t0 = int(tb[j]) + qd * 4
                nt4 = min(4, T_j[j] - qd * 4)
                # block [nt4, 128, 256] -> [p, i, (st, sl)]
                blk = Ehi[t0:t0 + nt4]             # [nt4, 128, 256]
                bt = blk.reshape(nt4, 128, 2, 128).transpose(3, 2, 0, 1)
                emT8[:, g, :, 0:nt4 * 128] = bt.reshape(128, 2, nt4 * 128)
                ql = qloc[t0:t0 + nt4]             # [nt4, 128]
                tt, pp = np.nonzero(ql >= 0)
                q1hot[ql[tt, pp], g, tt * 128 + pp] = np.float32(1.0)

        # P8 [128, NT, 128] fp8: one-hot wm skeleton [slot, tile, seg]
        P8 = np.zeros((128, NT, 128), dtype=np8)
        ttn, ppn = np.nonzero(segl < 255)
        P8[ppn, ttn, segl[ttn, ppn]] = np.float32(1.0)

        # C8 [128, NW, 2(hh), 128]: fp8 c bias rows, k-padded to 128.
        # Rows 0:16 carry c_hi, rows 16:32 the fp8 residual c_lo, so the
        # bias matmul streams fp8 at 2 elem/cycle with ~fp16 precision.
        C8 = np.zeros((128, NW, 2, 128), dtype=np8)
        for j in range(NW):
            qg = qbase[j] + np.arange(16) + c * QPC
            ok = qg < (c + 1) * QPC
            qgc = np.minimum(qg, (c + 1) * QPC - 1)
            chs = np.where(ok[:, None], c_hi[qgc], 0.0)
            cls = np.where(ok[:, None], c_lo[qgc], 0.0)
            C8[0:16, j, 0, :] = chs[:, 0:128].astype(np8)
            C8[0:16, j, 1, :] = chs[:, 128:256].astype(np8)
            C8[16:32, j, 0, :] = cls[:, 0:128].astype(np8)
            C8[16:32, j, 1, :] = cls[:, 128:256].astype(np8)

        wq8 = np.zeros((128, 2, 2, 128), dtype=np8)
        for hh in range(2):
            for i in range(2):
                wq8[:, hh, i, :] = W1[i * 128:(i + 1) * 128,
                                      hh * 128:(hh + 1) * 128].astype(np8)

        v16 = np.zeros((128, 2), dtype=np.float16)
        v16[:, 0] = v_s[0:128].astype(np.float16)
        v16[:, 1] = v_s[128:256].astype(np.float16)

        im = {
            "em16": np.ascontiguousarray(em16.reshape(128, NT * 257)),
            "emT8": np.ascontiguousarray(emT8.reshape(128, QT * 2 * 512)),
            "p8": np.ascontiguousarray(P8.reshape(128, NT * 128)),
            "C8": np.ascontiguousarray(C8.reshape(128, NW * 2 * 128)),
            "q1hot2": np.ascontiguousarray(
                np.concatenate([q1hot, q1hot], axis=0).reshape(32, QT * 512)),
            "wq8": np.ascontiguousarray(wq8.reshape(128, 512)),
            "v16": v16,
        }
        in_maps.append(im)

    # ---------------- build the SPMD program ----------------
    print("[kernel] host prep done", flush=True)
    nc = bacc.Bacc("TRN2", target_bir_lowering=False, debug=False,
                   num_devices=NCORES)

    def din(name, shape, dt):
        return nc.dram_tensor(name, shape, dt, kind="ExternalInput").ap()

    emh_ap = din("em16", [128, NT * 257], f16)
    emt_ap = din("emT8", [128, QT * 2 * 512], f8)
    q1_ap = din("q1hot2", [32, QT * 512], f8)
    p8_ap = din("p8", [128, NT * 128], f8)
    c2_ap = din("C8", [128, NW * 2 * 128], f8)
    wq_ap = din("wq8", [128, 512], f8)
    v_ap = din("v16", [128, 2], f16)
    out_ap = nc.dram_tensor("out", [SPC, 257], f16, kind="ExternalOutput").ap()

    AGG16 = os.environ.get("KERNEL_AGG16", "0") == "1"

    import time as _time
    _t0 = _time.time()
    with tile.TileContext(nc) as tc, ExitStack() as ctx:
        cons = ctx.enter_context(tc.tile_pool(name="cons", bufs=1))
        emhp = ctx.enter_context(tc.tile_pool(name="emhp", bufs=4))
        emtp = ctx.enter_context(tc.tile_pool(name="emtp", bufs=4))
        q1p = ctx.enter_context(tc.tile_pool(name="q1p", bufs=1))
        p8p = ctx.enter_context(tc.tile_pool(name="p8p", bufs=4))
        htp = ctx.enter_context(tc.tile_pool(name="htp", bufs=6))
        ep = ctx.enter_context(tc.tile_pool(name="ep", bufs=3))
        wmp = ctx.enter_context(tc.tile_pool(name="wmp", bufs=3))
        op = ctx.enter_context(tc.tile_pool(name="op", bufs=3))
        psz = ctx.enter_context(tc.tile_pool(name="psz", bufs=2, space="PSUM"))
        psa = ctx.enter_context(tc.tile_pool(name="psa", bufs=2, space="PSUM"))
        pss = ctx.enter_context(tc.tile_pool(name="pss", bufs=2, space="PSUM"))

        def cload(tag, shape, dt, ap):
            t = cons.tile(shape, dt, tag=tag)
            nc.gpsimd.dma_start(t[:], ap[:])
            return t

        wq8 = cload("wq8", [128, 2, 2, 128], f8, wq_ap)    # [p, hh, i, m]
        c2 = cload("C8", [128, NW, 2, 128], f8, c2_ap)    # [p, j, hh, m]
        v16 = cload("v16", [128, 2], f16, v_ap)
        q1tiles = []
        for qi in range(4):
            q1t = q1p.tile([128, QMAXW, 512], f8, tag=f"q1t{qi}")
            nc.vector.memset(q1t[:], 0.0)
            q1tiles.append(q1t)

        NW_RUN = int(os.environ.get("KERNEL_NWIN", str(NW)))

        # state carried across the 1-window software pipeline lag
        prev = None  # (j, TW, agg_tile, wm_tile, emh_tile)

        loaded = {}

        def load_window(lj):
            lTW, lQW = T_j[lj], Q_j[lj]
            lbase, lqb = int(tb[lj]), int(qb[lj])
            emh = emhp.tile([128, TMAX, 257], f16, tag="emh")
            nc.sync.dma_start(
                emh[:, 0:lTW, :],
                emh_ap[:, lbase * 257:(lbase + lTW) * 257])
            emt = emtp.tile([128, QMAXW, 2, 512], f8, tag="emt")
            nc.sync.dma_start(
                emt[:, 0:lQW, :, :],
                emt_ap[:, lqb * 1024:(lqb + lQW) * 1024])
            q1 = q1tiles[lj % 4]
            nc.gpsimd.dma_start(
                q1[0:32, 0:lQW, :],
                q1_ap[:, lqb * 512:(lqb + lQW) * 512])
            p8w = p8p.tile([128, TMAX, 128], f8, tag="p8w")
            nc.sync.dma_start(
                p8w[:, 0:lTW, :],
                p8_ap[:, lbase * 128:(lbase + lTW) * 128])
            loaded[lj] = (emh, emt, q1, p8w)

        for j in range(NW_RUN + 1):
            if j < NW_RUN:
                TW = T_j[j]
                QW = Q_j[j]
                base = int(tb[j])
                qbase_j = int(qb[j])

                if j == 0:
                    load_window(0)
                if j + 1 < NW_RUN:
                    load_window(j + 1)
                emh, emt, q1, p8w = loaded.pop(j)

                scp = pss.tile([128, TMAX], f32, tag="sc")
                hts = []

                for qd in range(QW):
                    nt4 = min(4, TW - qd * 4)
                    ns = nt4 * 128
                    zp = psz.tile([128, 2, 512], f32, tag="z")
                    for hh in range(2):
                        nc.tensor.matmul(
                            zp[:, hh, 0:ns], wq8[:, hh, :, :],
                            emt[:, qd, :, 0:ns],
                            start=True, stop=False, perf_mode=DR)
                        nc.tensor.matmul(
                            zp[:, hh, 0:ns], c2[:, j, hh, :],
                            q1[:, qd, 0:ns],
                            start=False, stop=True)
                    ht = htp.tile([128, 2, 512], f16, tag="ht")
                    nc.scalar.activation(ht[:, :, 0:ns], zp[:, :, 0:ns],
                                         AF.Tanh)
                    hts.append((ht, nt4))

                    # interleave: agg chunk of previous window between this
                    # quad's z and the (lagged) score matmuls
                    if prev is not None and qd < len(prev[4]):
                        _agg_chunk(nc, prev, qd, AGG16, DR, f16)

                    if qd > 1:
                        _score_quad(nc, scp, hts[qd - 2][0], hts[qd - 2][1],
                                    qd - 2, v16)
                for qq in range(max(0, QW - 2), QW):
                    _score_quad(nc, scp, hts[qq][0], hts[qq][1], qq, v16)
                if prev is not None:
                    for qd in range(QW, len(prev[4])):
                        _agg_chunk(nc, prev, qd, AGG16, DR, f16)

                e_sb = ep.tile([128, 2 * PMAXW], f32, tag="e")
                nc.scalar.activation(e_sb[:, 0:TW], scp[:, 0:TW], AF.Exp)

                wm = wmp.tile([128, TMAX, 128], f16, tag="wm")
                nc.vector.tensor_tensor(
                    wm[:, 0:TW, :], p8w[:, 0:TW, :],
                    e_sb[:, 0:TW, None].broadcast_to([128, TW, 128]),
                    OP.mult)

                agg = psa.tile([128, 257], f32, tag="agg")
                chunks = [min(4, TW - qd * 4) for qd in range(QW)]
                cur = (j, TW, agg, wm, chunks, emh)
            else:
                cur = None
                if prev is not None:
                    for qd in range(len(prev[4])):
                        _agg_chunk(nc, prev, qd, AGG16, DR, f16)

            if prev is not None:
                # drain previous window: copy agg|den to fp16, DMA out
                pj = prev[0]
                out_sb = op.tile([128, 257], f16, tag="out")
                nc.vector.tensor_copy(out_sb[:], prev[2][:])
                nc.gpsimd.dma_start(
                    out_ap[pj * 128:(pj + 1) * 128, :], out_sb[:])

            prev = cur

    print(f"[kernel] program built+scheduled in {_time.time()-_t0:.1f}s",
          flush=True)
    nc.compile()
    print("[kernel] bacc.compile done; launching", flush=True)

    def assemble(core_outs):
        full = np.empty((B, SEQ_LEN, 3 * H), dtype=np.float32)
        fl = full.reshape(NUM_SEG, 3 * H)
        mask = (counts_all > 0)
        for c in range(NCORES):
            o = np.asarray(core_outs[c], dtype=np.float32)  # [SPC, 257]
            blk = fl[c * SPC:(c + 1) * SPC]
            for j in range(NW):
                w = perm[c][j]
                rows = o[j * 128:(j + 1) * 128]
                den = rows[:, 256:257]
                np.divide(rows[:, 0:256], den,
                          out=blk[w * WIN:(w + 1) * WIN, 0:256],
                          where=den > 0)
                blk[w * WIN:(w + 1) * WIN, 0:256][den[:, 0] <= 0] = 0.0
        segq = np.arange(NUM_SEG) // SEQ_LEN
        sm = mask[:, None].astype(np.float32)
        fl[:, 256:512] = ent_embeds[s[segq]] * sm
        fl[:, 512:768] = rel_embeds[r[segq]] * sm
        return full

    if os.environ.get("KERNEL_SIM"):
        from concourse.bass_interp import CoreSim
        sim = CoreSim(nc, trace=False)
        for k, v in in_maps[0].items():
            sim.tensor(k)[:] = v
        sim.simulate(check_with_hw=False)
        print("[kernel] CoreSim passed", flush=True)
        return assemble([np.array(sim.tensor("out"))] * NCORES)

    trace = bool(int(os.environ.get("KERNEL_TRACE", "0")))
    if trace:
        _install_prof_hook()
    res = run_bass_kernel_spmd(nc, in_maps, list(range(NCORES)), trace=trace)
    if trace and res.exec_time_ns is not None:
        print(f"HW exec time: {res.exec_time_ns} ns")

    return assemble([res.results[c]["out"] for c in range(NCORES)])


def _score_quad(nc, scp, ht, nt4, qd, v16):
    for tt in range(nt4):
        t = qd * 4 + tt
        for hh in range(2):
            nc.tensor.matmul(
                scp[:, t:t + 1],
                ht[:, hh, tt * 128:(tt + 1) * 128],
                v16[:, hh:hh + 1],
                start=(hh == 0), stop=(hh == 1))


def _agg_chunk(nc, prev, qd, AGG16, DR, f16dt):
    _, TW, agg, wm, chunks, emh = prev
    nt4 = chunks[qd]
    for tt in range(nt4):
        t = qd * 4 + tt
        nc.tensor.matmul(
            agg[:], wm[:, t, :], emh[:, t, :],
            start=(t == 0), stop=(t == TW - 1))


def _install_prof_hook():
    """Shim antenv.axon_hooks so trace=True can NTFF-profile under axon."""
    import contextlib
    import ctypes
    import types

    import antenv

    if "antenv.axon_hooks" in sys.modules:
        return
    so = "/opt/axon/libaxon_pjrt.so"
    lib = ctypes.CDLL(so)
    if not hasattr(lib, "axon_start_nrt_profile"):
        return
    lib.axon_start_nrt_profile.argtypes = [ctypes.POINTER(ctypes.c_int64),
                                           ctypes.c_size_t]
    lib.axon_start_nrt_profile.restype = ctypes.c_int64
    lib.axon_stop_nrt_profile.argtypes = [ctypes.c_char_p]
    lib.axon_stop_nrt_profile.restype = ctypes.c_int64

    @contextlib.contextmanager
    def _hook(output_dir, device_ids):
        import jax

        jax.devices()
        if device_ids:
            ids = (ctypes.c_int64 * len(device_ids))(*device_ids)
            rc = lib.axon_start_nrt_profile(ids, len(device_ids))
        else:
            rc = lib.axon_start_nrt_profile(None, 0)
        if rc != 0:
            raise RuntimeError(f"axon_start_nrt_profile rc={rc}")
        try:
            yield
        finally:
            n = lib.axon_stop_nrt_profile(str(output_dir).encode())
            print(f"profile: {n} file(s) written to {output_dir}",
                  file=sys.stderr)

    mod = types.ModuleType("antenv.axon_hooks")
    mod.get_axon_ntff_profile_hook = lambda: _hook
    mod.set_axon_ntff_profile_hook = lambda h: None
    sys.modules["antenv.axon_hooks"] = mod
    antenv.axon_hooks = mod
